# revision 8
# baseline (speedup 1.0000x reference)
"""Multi-head causal attention (QKV proj + attention + out proj) on 8 TRN2
NeuronCores.

Sharding: 2-way data-parallel over batch x 4-way tensor-parallel over heads
(Megatron-style).  Core c handles batch c//4 and heads [4*(c%4), 4*(c%4)+4).
Each core computes its 4 heads' Q/K/V projections (column-parallel), the
attention for those heads, and a partial output projection (row-parallel).
The host sums the 4 TP partials per batch and adds the output bias.

Device layout notes (per core):
  - Q^T/K^T kept "d-major": [f=256 on partitions as 2 blocks of 128, S free].
    Weights are fed pre-transposed from the host so no on-chip transposes are
    needed; the softmax scale 1/sqrt(D) is folded into Wq/bq on the host.
  - scores^T[k, q] blocks [128, 512] = K_h^T-tile.T-matmul; exp on ACT;
    causal diagonal blocks multiplied by a binary mask (4 static tiles).
  - P^T @ [1|V|1] accumulated on PE per q-chunk: V carries ones columns on
    both sides so the same matmul emits the softmax denominator row, at a
    partition offset such that all downstream ops are partition-aligned for
    both even heads (rows 0..64) and odd heads (rows 63..127).
  - normalization: reciprocal of denom row, PE-broadcast to 64 partitions,
    single tensor_mult -> O^T; out-proj consumes O^T directly.
  - matmuls run as float32r (full-rate fp32 mode) via bitcast.
"""

import numpy as np
from contextlib import ExitStack

import concourse.bass as bass
import concourse.mybir as mybir
import concourse.tile as tile
from concourse import bacc
from concourse.bass import ds
from concourse.bass_utils import run_bass_kernel_spmd

B, S_FULL, E, H = 2, 2048, 1024, 16
D = E // H          # 64
NCORES = 8
TP = 4              # tensor-parallel ways (over heads)
HL = H // TP        # 4 local heads per core
F = HL * D          # 256 local projection width
P = 128
QCH = 512           # q-chunk / matmul moving-dim size
FP32 = mybir.dt.float32
F32R = mybir.dt.float32r
AF = mybir.ActivationFunctionType


def build(S=S_FULL, causal=True):
    ET = E // P          # 8 contraction tiles for projections
    NQ = S // QCH        # q chunks
    KT = S // P          # k tiles
    KPQ = QCH // P       # k tiles per q chunk (4)

    nc = bacc.Bacc()

    def din(name, shape, dt=FP32):
        return nc.declare_dram_parameter(name, shape, dt, isOutput=False)

    xqT = din("xqT", [E, S], F32R)
    xkT = din("xkT", [E, S], F32R)
    xvT = din("xvT", [E, S], F32R)
    wqT = din("wqT", [E, F], F32R)
    wkT = din("wkT", [E, F], F32R)
    wvT = din("wvT", [E, F], F32R)
    bq2 = din("bq2", [P, F // P])
    bk2 = din("bk2", [P, F // P])
    bvb = din("bvb", [P, F])
    woT = din("woT", [F, E], F32R)
    msk = din("msk", [P, KPQ, QCH], F32R)
    outT = nc.declare_dram_parameter("outT", [E, S], FP32, isOutput=True)

    with ExitStack() as ctx:
        ctx.enter_context(
            nc.allow_low_precision(reason="float32r is the intended matmul input dtype")
        )
        tc = ctx.enter_context(tile.TileContext(nc))
        const = ctx.enter_context(tc.tile_pool(name="const", bufs=1))
        xp = ctx.enter_context(tc.tile_pool(name="xp", bufs=3))
        pex = ctx.enter_context(tc.tile_pool(name="pex", bufs=4))
        prn = ctx.enter_context(tc.tile_pool(name="prn", bufs=2))
        opool = ctx.enter_context(tc.tile_pool(name="op", bufs=3))
        pp = ctx.enter_context(tc.tile_pool(name="pp", bufs=2, space="PSUM"))
        psc = ctx.enter_context(tc.tile_pool(name="psc", bufs=2, space="PSUM"))
        po = ctx.enter_context(tc.tile_pool(name="po", bufs=2, space="PSUM"))
        pb = ctx.enter_context(tc.tile_pool(name="pb", bufs=1, space="PSUM"))

        # ---- constants / persistent tensors ----
        wq_sb = const.tile([P, ET, F], F32R)
        nc.sync.dma_start(out=wq_sb, in_=wqT[:, :].rearrange("(t p) f -> p t f", p=P))
        wk_sb = const.tile([P, ET, F], F32R)
        nc.sync.dma_start(out=wk_sb, in_=wkT[:, :].rearrange("(t p) f -> p t f", p=P))
        wv_sb = const.tile([P, ET, F], F32R)
        nc.sync.dma_start(out=wv_sb, in_=wvT[:, :].rearrange("(t p) f -> p t f", p=P))
        wo_sb = const.tile([P, F // P, E], F32R)
        nc.sync.dma_start(out=wo_sb, in_=woT[:, :].rearrange("(b p) e -> p b e", p=P))
        bq_sb = const.tile([P, F // P], FP32)
        nc.sync.dma_start(out=bq_sb, in_=bq2[:, :])
        bk_sb = const.tile([P, F // P], FP32)
        nc.sync.dma_start(out=bk_sb, in_=bk2[:, :])
        bvb_sb = const.tile([P, F], FP32)
        nc.sync.dma_start(out=bvb_sb, in_=bvb[:, :])
        msk_sb = const.tile([P, KPQ, QCH], F32R)
        nc.sync.dma_start(out=msk_sb, in_=msk[:, :, :])
        # memset cannot emit float32r (walrus ISA check); go via an FP32
        # scratch + ACT copy, which is a legal f32r producer.
        ones_f32 = const.tile([P, D], FP32)
        nc.vector.memset(ones_f32, 1.0)
        ones_sb = const.tile([P, D], F32R)
        nc.scalar.activation(ones_sb, ones_f32, AF.Copy)

        qT_sb = const.tile([P, F // P, S], F32R)
        kT_sb = const.tile([P, F // P, S], F32R)
        # V with a trailing ones column: AV matmul emits the softmax
        # denominator as PSUM row D for free.
        vo_sb = const.tile([P, KT, HL, D + 1], F32R)
        nc.scalar.activation(
            vo_sb[:, :, :, D:D + 1],
            ones_f32[:, 0:KT * HL].rearrange("p (a b c) -> p a b c", a=KT, b=HL, c=1),
            AF.Copy,
        )
        oT_sb = const.tile([P, F // P, S], F32R)

        # ---- projections ----
        for j in range(NQ):
            for (xT, w_sb, b_sb, dst) in (
                (xqT, wq_sb, bq_sb, qT_sb),
                (xkT, wk_sb, bk_sb, kT_sb),
            ):
                xt = xp.tile([P, ET, QCH], F32R, tag="xt")
                nc.sync.dma_start(
                    out=xt,
                    in_=xT[:, :].rearrange("(t p) s -> p t s", p=P)[:, :, ds(j * QCH, QCH)],
                )
                for blk in range(F // P):
                    acc = pp.tile([P, QCH], FP32, tag="acc")
                    for et in range(ET):
                        nc.tensor.matmul(
                            acc,
                            w_sb[:, et, ds(blk * P, P)],
                            xt[:, et, :],
                            start=(et == 0),
                            stop=(et == ET - 1),
                        )
                    nc.vector.tensor_scalar_add(
                        dst[:, blk, ds(j * QCH, QCH)], acc, b_sb[:, blk:blk + 1]
                    )
            # V projection in s-major layout, written between the ones columns
            xt = xp.tile([P, ET, QCH], F32R, tag="xt")
            nc.sync.dma_start(
                out=xt,
                in_=xvT[:, :].rearrange("(t p) s -> p t s", p=P)[:, :, ds(j * QCH, QCH)],
            )
            for sl in range(QCH // P):
                st = j * (QCH // P) + sl
                acc = pp.tile([P, QCH], FP32, tag="acc")
                for et in range(ET):
                    nc.tensor.matmul(
                        acc[:, 0:F],
                        xt[:, et, ds(sl * P, P)],
                        wv_sb[:, et, :],
                        start=(et == 0),
                        stop=(et == ET - 1),
                    )
                for h in range(HL):
                    nc.vector.tensor_add(
                        vo_sb[:, st, h, 0:D],
                        acc[:, ds(h * D, D)],
                        bvb_sb[:, ds(h * D, D)],
                    )

        # ---- attention ----
        for j in range(NQ):
            for h in range(HL):
                blkh = h // 2
                doff = (h % 2) * D          # partition offset of head h in blk

                nkt = KPQ * (j + 1) if causal else KT
                po_t = po.tile([P, QCH], FP32, tag="po")
                for kt in range(nkt):
                    sc = psc.tile([P, QCH], FP32, tag="sc")
                    nc.tensor.matmul(
                        sc,
                        kT_sb[doff:doff + D, blkh, ds(kt * P, P)],
                        qT_sb[doff:doff + D, blkh, ds(j * QCH, QCH)],
                        start=True,
                        stop=True,
                    )
                    pt = pex.tile([P, QCH], F32R, tag="pt")
                    nc.scalar.activation(pt, sc, AF.Exp)
                    if causal and kt >= KPQ * j:
                        t = kt - KPQ * j
                        nc.vector.tensor_mul(pt, pt, msk_sb[:, t, :])
                    nc.tensor.matmul(
                        po_t[0:D + 1, :],
                        vo_sb[:, kt, h, :],
                        pt,
                        start=(kt == 0),
                        stop=(kt == nkt - 1),
                    )
                # normalization: O^T rows 0..D-1, denominator row D
                rc = prn.tile([P, QCH], F32R, tag="rc")
                nc.vector.reciprocal(rc[D:D + 1, :], po_t[D:D + 1, :])
                bc = pb.tile([P, QCH], FP32, tag="bc")
                nc.tensor.matmul(
                    bc[0:D, :],
                    ones_sb[D:D + 1, :],
                    rc[D:D + 1, :],
                    start=True,
                    stop=True,
                )
                bcs = prn.tile([P, QCH], FP32, tag="bcs")
                nc.scalar.activation(bcs[0:D, :], bc[0:D, :], AF.Copy)
                nc.vector.tensor_mul(
                    oT_sb[doff:doff + D, blkh, ds(j * QCH, QCH)],
                    po_t[0:D, :],
                    bcs[0:D, :],
                )

        # ---- output projection (partial over local heads) ----
        for eb in range(E // P):
            for j in range(NQ):
                acc = pp.tile([P, QCH], FP32, tag="acc")
                for fb in range(F // P):
                    nc.tensor.matmul(
                        acc,
                        wo_sb[:, fb, ds(eb * P, P)],
                        oT_sb[:, fb, ds(j * QCH, QCH)],
                        start=(fb == 0),
                        stop=(fb == F // P - 1),
                    )
                ot = opool.tile([P, QCH], FP32, tag="ot")
                nc.vector.tensor_copy(ot, acc)
                nc.sync.dma_start(
                    out=outT[ds(eb * P, P), ds(j * QCH, QCH)], in_=ot
                )

    nc.compile()
    return nc


def make_masks(S=S_FULL):
    KPQ = QCH // P
    m = np.zeros((P, KPQ, QCH), np.float32)
    for t in range(KPQ):
        kk = np.arange(P)[:, None]
        qq = np.arange(QCH)[None, :]
        m[:, t, :] = (qq >= kk + P * t).astype(np.float32)
    return m


def make_in_maps(query, key, value, Wq, bq, Wk, bk, Wv, bv, Wo, bo, S=S_FULL):
    scale = float(D) ** -0.5
    q = np.asarray(query, np.float32)
    k = np.asarray(key, np.float32)
    v = np.asarray(value, np.float32)
    Wq = np.asarray(Wq, np.float32)
    Wk = np.asarray(Wk, np.float32)
    Wv = np.asarray(Wv, np.float32)
    Wo = np.asarray(Wo, np.float32)
    bq = np.asarray(bq, np.float32)
    bk = np.asarray(bk, np.float32)
    bv = np.asarray(bv, np.float32)
    masks = make_masks(S)
    in_maps = []
    for c in range(NCORES):
        b, tp = divmod(c, TP)
        rows = slice(tp * F, (tp + 1) * F)
        in_maps.append({
            "xqT": np.ascontiguousarray(q[b].T),
            "xkT": np.ascontiguousarray(k[b].T),
            "xvT": np.ascontiguousarray(v[b].T),
            "wqT": np.ascontiguousarray((Wq[rows] * scale).T),
            "wkT": np.ascontiguousarray(Wk[rows].T),
            "wvT": np.ascontiguousarray(Wv[rows].T),
            "bq2": np.ascontiguousarray((bq[rows] * scale).reshape(F // P, P).T),
            "bk2": np.ascontiguousarray(bk[rows].reshape(F // P, P).T),
            "bvb": np.ascontiguousarray(np.broadcast_to(bv[rows], (P, F))),
            "woT": np.ascontiguousarray(Wo[:, rows].T),
            "msk": masks,
        })
    return in_maps


_CACHE = {}


def _get_nc(causal):
    if causal not in _CACHE:
        _CACHE[causal] = build(S_FULL, causal)
    return _CACHE[causal]


def kernel(query, key, value, Wq, bq, Wk, bk, Wv, bv, Wo, bo, is_causal):
    causal = bool(int(np.asarray(is_causal)))
    nc = _get_nc(causal)
    in_maps = make_in_maps(query, key, value, Wq, bq, Wk, bk, Wv, bv, Wo, bo)
    res = run_bass_kernel_spmd(nc, in_maps, core_ids=list(range(NCORES)))
    out = np.zeros((B, S_FULL, E), np.float32)
    for c in range(NCORES):
        b, tp = divmod(c, TP)
        out[b] += res.results[c]["outT"].T
    out += np.asarray(bo, np.float32)
    return out


# revision 13
# speedup vs baseline: 1.0023x; 1.0023x over previous
"""Multi-head causal attention (QKV proj + attention + out proj) on 8 TRN2
NeuronCores.

Sharding: 2-way data-parallel over batch x 4-way tensor-parallel over heads
(Megatron-style).  Core c handles batch c//4 and heads [4*(c%4), 4*(c%4)+4).
Each core computes its 4 heads' Q/K/V projections (column-parallel), the
attention for those heads, and a partial output projection (row-parallel).
The host sums the 4 TP partials per batch and adds the output bias.

Device layout notes (per core):
  - Q^T/K^T kept "d-major": [f=256 on partitions as 2 blocks of 128, S free].
    Weights are fed pre-transposed from the host so no on-chip transposes are
    needed; the softmax scale 1/sqrt(D) is folded into Wq/bq on the host.
  - scores^T[k, q] blocks [128, 512] = K_h^T-tile.T-matmul; exp on ACT;
    causal diagonal blocks multiplied by a binary mask (4 static tiles).
  - P^T @ [1|V|1] accumulated on PE per q-chunk: V carries ones columns on
    both sides so the same matmul emits the softmax denominator row, at a
    partition offset such that all downstream ops are partition-aligned for
    both even heads (rows 0..64) and odd heads (rows 63..127).
  - normalization: reciprocal of denom row, PE-broadcast to 64 partitions,
    single tensor_mult -> O^T; out-proj consumes O^T directly.
  - matmuls run as float32r (full-rate fp32 mode) via bitcast.
"""

import numpy as np
from contextlib import ExitStack

import concourse.bass as bass
import concourse.mybir as mybir
import concourse.tile as tile
from concourse import bacc
from concourse.bass import ds
from concourse.bass_utils import run_bass_kernel_spmd

B, S_FULL, E, H = 2, 2048, 1024, 16
D = E // H          # 64
NCORES = 8
TP = 4              # tensor-parallel ways (over heads)
HL = H // TP        # 4 local heads per core
F = HL * D          # 256 local projection width
P = 128
QCH = 512           # q-chunk / matmul moving-dim size
FP32 = mybir.dt.float32
F32R = mybir.dt.float32r
AF = mybir.ActivationFunctionType


def build(S=S_FULL, causal=True):
    ET = E // P          # 8 contraction tiles for projections
    NQ = S // QCH        # q chunks
    KT = S // P          # k tiles
    KPQ = QCH // P       # k tiles per q chunk (4)

    nc = bacc.Bacc()

    def din(name, shape, dt=FP32):
        return nc.declare_dram_parameter(name, shape, dt, isOutput=False)

    xqT = din("xqT", [E, S], F32R)
    xkT = din("xkT", [E, S], F32R)
    xvT = din("xvT", [E, S], F32R)
    wqT = din("wqT", [E, F], F32R)
    wkT = din("wkT", [E, F], F32R)
    wvT = din("wvT", [E, F], F32R)
    bq2 = din("bq2", [P, F // P])
    bk2 = din("bk2", [P, F // P])
    bvb = din("bvb", [P, F])
    woT = din("woT", [F, E], F32R)
    msk = din("msk", [P, KPQ, QCH], F32R)
    outT = nc.declare_dram_parameter("outT", [E, S], FP32, isOutput=True)

    with ExitStack() as ctx:
        ctx.enter_context(
            nc.allow_low_precision(reason="float32r is the intended matmul input dtype")
        )
        tc = ctx.enter_context(tile.TileContext(nc))
        const = ctx.enter_context(tc.tile_pool(name="const", bufs=1))
        xp = ctx.enter_context(tc.tile_pool(name="xp", bufs=3))
        pex = ctx.enter_context(tc.tile_pool(name="pex", bufs=4))
        prn = ctx.enter_context(tc.tile_pool(name="prn", bufs=2))
        opool = ctx.enter_context(tc.tile_pool(name="op", bufs=3))
        pp = ctx.enter_context(tc.tile_pool(name="pp", bufs=2, space="PSUM"))
        psc = ctx.enter_context(tc.tile_pool(name="psc", bufs=2, space="PSUM"))
        po = ctx.enter_context(tc.tile_pool(name="po", bufs=3, space="PSUM"))
        pb = ctx.enter_context(tc.tile_pool(name="pb", bufs=1, space="PSUM"))

        # ---- constants / persistent tensors ----
        # masks first: the PE warm-up matmuls below depend only on this
        # small DMA, so the PE clock ramps while the big loads stream in.
        msk_sb = const.tile([P, KPQ, QCH], F32R)
        nc.sync.dma_start(out=msk_sb, in_=msk[:, :, :])
        wq_sb = const.tile([P, ET, F], F32R)
        nc.sync.dma_start(out=wq_sb, in_=wqT[:, :].rearrange("(t p) f -> p t f", p=P))
        wk_sb = const.tile([P, ET, F], F32R)
        nc.sync.dma_start(out=wk_sb, in_=wkT[:, :].rearrange("(t p) f -> p t f", p=P))
        wv_sb = const.tile([P, ET, F], F32R)
        nc.sync.dma_start(out=wv_sb, in_=wvT[:, :].rearrange("(t p) f -> p t f", p=P))
        wo_sb = const.tile([P, F // P, E], F32R)
        nc.sync.dma_start(out=wo_sb, in_=woT[:, :].rearrange("(b p) e -> p b e", p=P))
        bq_sb = const.tile([P, F // P], FP32)
        nc.sync.dma_start(out=bq_sb, in_=bq2[:, :])
        bk_sb = const.tile([P, F // P], FP32)
        nc.sync.dma_start(out=bk_sb, in_=bk2[:, :])
        bvb_sb = const.tile([P, F], FP32)
        nc.sync.dma_start(out=bvb_sb, in_=bvb[:, :])
        # PE clock warm-up: ~28 back-to-back dummy matmuls (WAW-serialized on
        # one PSUM tile) keep the tensor engine busy through the HAM window
        # while the input DMAs stream, so real work starts at 2.4 GHz.
        wps = pb.tile([P, QCH], FP32, tag="bc")
        for _ in range(28):
            nc.tensor.matmul(
                wps, msk_sb[:, 0, 0:P], msk_sb[:, 0, :], start=True, stop=True
            )
        # memset cannot emit float32r (walrus ISA check); go via an FP32
        # scratch + ACT copy, which is a legal f32r producer.
        ones_f32 = const.tile([P, D], FP32)
        nc.vector.memset(ones_f32, 1.0)
        ones_sb = const.tile([P, D], F32R)
        nc.scalar.activation(ones_sb, ones_f32, AF.Copy)

        qT_sb = const.tile([P, F // P, S], F32R)
        kT_sb = const.tile([P, F // P, S], F32R)
        # V with a trailing ones column: AV matmul emits the softmax
        # denominator as PSUM row D for free.
        vo_sb = const.tile([P, KT, HL, D + 1], F32R)
        nc.scalar.activation(
            vo_sb[:, :, :, D:D + 1],
            ones_f32[:, 0:KT * HL].rearrange("p (a b c) -> p a b c", a=KT, b=HL, c=1),
            AF.Copy,
        )
        oT_sb = const.tile([P, F // P, S], F32R)

        # ---- projections ----
        for j in range(NQ):
            for (xT, w_sb, b_sb, dst) in (
                (xqT, wq_sb, bq_sb, qT_sb),
                (xkT, wk_sb, bk_sb, kT_sb),
            ):
                xt = xp.tile([P, ET, QCH], F32R, tag="xt")
                nc.sync.dma_start(
                    out=xt,
                    in_=xT[:, :].rearrange("(t p) s -> p t s", p=P)[:, :, ds(j * QCH, QCH)],
                )
                for blk in range(F // P):
                    acc = pp.tile([P, QCH], FP32, tag="acc")
                    for et in range(ET):
                        nc.tensor.matmul(
                            acc,
                            w_sb[:, et, ds(blk * P, P)],
                            xt[:, et, :],
                            start=(et == 0),
                            stop=(et == ET - 1),
                        )
                    nc.vector.tensor_scalar_add(
                        dst[:, blk, ds(j * QCH, QCH)], acc, b_sb[:, blk:blk + 1]
                    )
            # V projection in s-major layout, written between the ones columns
            xt = xp.tile([P, ET, QCH], F32R, tag="xt")
            nc.sync.dma_start(
                out=xt,
                in_=xvT[:, :].rearrange("(t p) s -> p t s", p=P)[:, :, ds(j * QCH, QCH)],
            )
            for sl in range(QCH // P):
                st = j * (QCH // P) + sl
                acc = pp.tile([P, QCH], FP32, tag="acc")
                for et in range(ET):
                    nc.tensor.matmul(
                        acc[:, 0:F],
                        xt[:, et, ds(sl * P, P)],
                        wv_sb[:, et, :],
                        start=(et == 0),
                        stop=(et == ET - 1),
                    )
                for h in range(HL):
                    nc.vector.tensor_add(
                        vo_sb[:, st, h, 0:D],
                        acc[:, ds(h * D, D)],
                        bvb_sb[:, ds(h * D, D)],
                    )

        # ---- attention ----
        # Normalization of head (j,h) is emitted AFTER the next head's
        # matmul block: the slow single-partition reciprocal (≈3.4us on DVE)
        # otherwise stalls the in-order PE stream right before the bc
        # broadcast matmul and re-throttles the PE clock.
        def emit_normalize(j, h, po_t):
            blkh = h // 2
            doff = (h % 2) * D
            rc = prn.tile([P, QCH], F32R, tag="rc")
            nc.vector.reciprocal(rc[D:D + 1, :], po_t[D:D + 1, :])
            bc = pb.tile([P, QCH], FP32, tag="bc")
            nc.tensor.matmul(
                bc[0:D, :],
                ones_sb[D:D + 1, :],
                rc[D:D + 1, :],
                start=True,
                stop=True,
            )
            bcs = prn.tile([P, QCH], FP32, tag="bcs")
            nc.scalar.activation(bcs[0:D, :], bc[0:D, :], AF.Copy)
            nc.vector.tensor_mul(
                oT_sb[doff:doff + D, blkh, ds(j * QCH, QCH)],
                po_t[0:D, :],
                bcs[0:D, :],
            )

        pending = None
        for j in range(NQ):
            for h in range(HL):
                blkh = h // 2
                doff = (h % 2) * D          # partition offset of head h in blk

                nkt = KPQ * (j + 1) if causal else KT
                po_t = po.tile([P, QCH], FP32, tag="po")
                for kt in range(nkt):
                    sc = psc.tile([P, QCH], FP32, tag="sc")
                    nc.tensor.matmul(
                        sc,
                        kT_sb[doff:doff + D, blkh, ds(kt * P, P)],
                        qT_sb[doff:doff + D, blkh, ds(j * QCH, QCH)],
                        start=True,
                        stop=True,
                    )
                    pt = pex.tile([P, QCH], F32R, tag="pt")
                    nc.scalar.activation(pt, sc, AF.Exp)
                    if causal and kt >= KPQ * j:
                        t = kt - KPQ * j
                        nc.vector.tensor_mul(pt, pt, msk_sb[:, t, :])
                    nc.tensor.matmul(
                        po_t[0:D + 1, :],
                        vo_sb[:, kt, h, :],
                        pt,
                        start=(kt == 0),
                        stop=(kt == nkt - 1),
                    )
                if pending is not None:
                    emit_normalize(*pending)
                pending = (j, h, po_t)
        emit_normalize(*pending)

        # ---- output projection (partial over local heads) ----
        for eb in range(E // P):
            for j in range(NQ):
                acc = pp.tile([P, QCH], FP32, tag="acc")
                for fb in range(F // P):
                    nc.tensor.matmul(
                        acc,
                        wo_sb[:, fb, ds(eb * P, P)],
                        oT_sb[:, fb, ds(j * QCH, QCH)],
                        start=(fb == 0),
                        stop=(fb == F // P - 1),
                    )
                ot = opool.tile([P, QCH], FP32, tag="ot")
                nc.vector.tensor_copy(ot, acc)
                nc.sync.dma_start(
                    out=outT[ds(eb * P, P), ds(j * QCH, QCH)], in_=ot
                )

    nc.compile()
    return nc


def make_masks(S=S_FULL):
    KPQ = QCH // P
    m = np.zeros((P, KPQ, QCH), np.float32)
    for t in range(KPQ):
        kk = np.arange(P)[:, None]
        qq = np.arange(QCH)[None, :]
        m[:, t, :] = (qq >= kk + P * t).astype(np.float32)
    return m


def make_in_maps(query, key, value, Wq, bq, Wk, bk, Wv, bv, Wo, bo, S=S_FULL):
    scale = float(D) ** -0.5
    q = np.asarray(query, np.float32)
    k = np.asarray(key, np.float32)
    v = np.asarray(value, np.float32)
    Wq = np.asarray(Wq, np.float32)
    Wk = np.asarray(Wk, np.float32)
    Wv = np.asarray(Wv, np.float32)
    Wo = np.asarray(Wo, np.float32)
    bq = np.asarray(bq, np.float32)
    bk = np.asarray(bk, np.float32)
    bv = np.asarray(bv, np.float32)
    masks = make_masks(S)
    in_maps = []
    for c in range(NCORES):
        b, tp = divmod(c, TP)
        rows = slice(tp * F, (tp + 1) * F)
        in_maps.append({
            "xqT": np.ascontiguousarray(q[b].T),
            "xkT": np.ascontiguousarray(k[b].T),
            "xvT": np.ascontiguousarray(v[b].T),
            "wqT": np.ascontiguousarray((Wq[rows] * scale).T),
            "wkT": np.ascontiguousarray(Wk[rows].T),
            "wvT": np.ascontiguousarray(Wv[rows].T),
            "bq2": np.ascontiguousarray((bq[rows] * scale).reshape(F // P, P).T),
            "bk2": np.ascontiguousarray(bk[rows].reshape(F // P, P).T),
            "bvb": np.ascontiguousarray(np.broadcast_to(bv[rows], (P, F))),
            "woT": np.ascontiguousarray(Wo[:, rows].T),
            "msk": masks,
        })
    return in_maps


_CACHE = {}


def _get_nc(causal):
    if causal not in _CACHE:
        _CACHE[causal] = build(S_FULL, causal)
    return _CACHE[causal]


def kernel(query, key, value, Wq, bq, Wk, bk, Wv, bv, Wo, bo, is_causal):
    causal = bool(int(np.asarray(is_causal)))
    nc = _get_nc(causal)
    in_maps = make_in_maps(query, key, value, Wq, bq, Wk, bk, Wv, bv, Wo, bo)
    res = run_bass_kernel_spmd(nc, in_maps, core_ids=list(range(NCORES)))
    out = np.zeros((B, S_FULL, E), np.float32)
    for c in range(NCORES):
        b, tp = divmod(c, TP)
        out[b] += res.results[c]["outT"].T
    out += np.asarray(bo, np.float32)
    return out
